# revision 50
# baseline (speedup 1.0000x reference)
"""Trainium2 Bass kernel for CFGSubASTExpressionCombiner (segment-softmax
attention over sub-ASTs grouped by PDG node).

Contract: kernel(**inputs) takes FULL unsharded numpy inputs, returns the
FULL [N_PDG, D] output. Internally shards PDG segments across 8 NeuronCores.
Within a core, segments are bin-packed into 49 blocks of <=128 segments so
per-block element counts are flat; per-block element-tile counts are baked
into the program.

Math (per segment s with element set E_s, all on device):
    q_s    = ast[root(s)]
    qk_s   = Wk q_s * scale                  (qkT [d, s] per block via PE)
    S[e,j] = x_e . qk_(lo+j)   j in [0,16)   (windowed: tile's segs in [lo,hi))
    P      = exp(S - 32*(j - slid_rel(e))^2)   (exact at j==slid; ~e^-32 off)
    U^T[d,s] += x^T P ; Z[s] += 1^T P        (PE, windowed col writes, PSUM)
    out_s  = (U_s @ Wv) / max(Z_s,eps)
No max-subtraction: scores are ~N(0,1), exp safe in fp32, softmax shift-inv.

v2 changes vs v1 (SWDGE desc-gen was 97% Pool-bound at 994ns/instr fixed):
  - gathers batched: 2 blocks of x-tiles per indirect DMA; all root rows in
    2 DMAs at start (fixed overhead amortized ~20x)
  - U accumulated TRANSPOSED (lhsT = gathered x directly) so the U matmul
    streams w<=16 cols instead of 257
  - true segment windows (w<=16) for S/exp/mask (free-dim writes only, no
    PSUM partition-offset constraint)
  - x transposes batched 4 tiles/PSUM bank; single DVE copy per batch
  - per-op engine assignment (DVE/Act/Pool) tuned via KNOBS
"""

import math

import numpy as np

import concourse.bass as bass
import concourse.bacc as bacc
import concourse.mybir as mybir
import concourse.tile as tile
from concourse.bass_utils import run_bass_kernel_spmd
from concourse.masks import make_identity

P = 128
D = 256
N_CORES = 8

# Full-problem constants (hardcoded per contract).
N_AST_FULL = 500000
N_PDG_FULL = 50000
SEGS_PER_CORE_FULL = N_PDG_FULL // N_CORES          # 6250
N_BLOCKS_FULL = math.ceil(SEGS_PER_CORE_FULL / P)   # 49

f32 = mybir.dt.float32
i32 = mybir.dt.int32
bf16 = mybir.dt.bfloat16

try:
    import ml_dtypes
    _NP_BF16 = ml_dtypes.bfloat16
except ImportError:
    _NP_BF16 = None
X_DT = bf16
X_NP_DT = _NP_BF16 if _NP_BF16 is not None else np.float32
if _NP_BF16 is None:
    X_DT = f32

EXP = mybir.ActivationFunctionType.Exp
COPY = mybir.ActivationFunctionType.Copy

# Engine assignment / batching knobs ("v"=DVE, "a"=Act, "p"=Pool).
# NOTE: Pool (GPSIMD) cannot access PSUM on real HW — only SBUF->SBUF ops
# (masks, memsets) may go to "p"; all PSUM->SBUF copies must be "v"/"a".
KNOBS = dict(
    xt_pattern="v",     # rotation over 4-tile transpose-copy groups
    qt_copy="a",
    qkt_copy="a",
    ut_copy="a",
    zrow_copy="a",
    o_scale="v",
    tb=3,               # tiles per transpose/exp batch (tb=4 miscomputes:
                        # grouped exp over a full 2KB PSUM bank breaks)
    gather_blocks=2,    # blocks of x-tiles per gather buffer
    nq=4,               # SWDGE queues to spread indirect gathers across
)


def _build_nc(n_ast, tiles_per_block, windows, reps=1, xdt=f32, knobs=None):
    """One SPMD NeuronCore program. Sizes fixed at build time.

    tiles_per_block[b] = number of 128-element tiles in segment-block b.
    windows[c] = (lo, w) local segment window of element-tile c.
    reps: repeat the whole block loop (differential timing only).
    """
    kn = dict(KNOBS)
    if knobs:
        kn.update(knobs)
    TB = kn["tb"]
    GB = kn["gather_blocks"]

    n_blocks = len(tiles_per_block)
    tile_off = np.concatenate([[0], np.cumsum(tiles_per_block)]).astype(int)
    n_cols = int(tile_off[-1])
    seg_slots = n_blocks * P
    e_slots = n_cols * P
    WMAX = max(w for _, w in windows)
    assert WMAX <= 32, WMAX
    assert all(lo + WMAX <= P for lo, _ in windows), "windows not clamped"
    nwrap = math.ceil(n_cols / 3)  # slxT wrap: tile c at base 32*(c%3)

    nc = bacc.Bacc(num_swdge_queues=max(1, kn["nq"]))
    ast = nc.declare_dram_parameter("ast", [n_ast, D], xdt, isOutput=False)
    wkt = nc.declare_dram_parameter("wkt", [D, D], xdt, isOutput=False)
    wv = nc.declare_dram_parameter("wv", [D, D], xdt, isOutput=False)
    gidx = nc.declare_dram_parameter("gidx", [e_slots], i32, isOutput=False)
    # transposed mask-quadratic rows [s2_hi, s2_lo, s, 1, 1] per tile
    slxq = nc.declare_dram_parameter("slxq", [15, nwrap * P], xdt, isOutput=False)
    rext = nc.declare_dram_parameter("rext", [P, WMAX], xdt, isOutput=False)
    root = nc.declare_dram_parameter("root", [seg_slots], i32, isOutput=False)
    out = nc.declare_dram_parameter("out", [seg_slots, D], f32, isOutput=True)

    def eng(code):
        return {"v": nc.vector, "a": nc.scalar, "p": nc.gpsimd}[code]

    _gq = [0]

    def indirect_gather(out_ap, offset_ap):
        """One 128-row indirect gather; HW supports exactly one index per
        partition per instruction. Rotate across SWDGE queues so Q7
        descriptor generation parallelizes."""
        inst = nc.gpsimd.indirect_dma_start(
            out=out_ap, out_offset=None, in_=ast[:],
            in_offset=bass.IndirectOffsetOnAxis(ap=offset_ap, axis=0),
        )
        q = _gq[0] % max(1, kn["nq"])
        _gq[0] += 1
        if q:
            inst.ins.queue = f"qPoolDynamic{q}"
        return inst

    def copy(code, out_ap, in_ap):
        if code == "a":
            nc.scalar.activation(out_ap, in_ap, COPY)
        else:
            eng(code).tensor_copy(out_ap, in_ap)

    with tile.TileContext(nc) as tc:
        with (
            tc.tile_pool(name="const", bufs=1) as cpool,
            tc.tile_pool(name="blk", bufs=3) as bpool,
            tc.tile_pool(name="xg", bufs=4) as xgpool,
            tc.tile_pool(name="xt", bufs=3 * TB) as xtpool,
            tc.tile_pool(name="et", bufs=4) as etpool,
            tc.tile_pool(name="ptr", bufs=2, space="PSUM") as ptr,
            tc.tile_pool(name="pq", bufs=2, space="PSUM") as pq,
            tc.tile_pool(name="pqt", bufs=1, space="PSUM") as pqt,
            tc.tile_pool(name="pu", bufs=2, space="PSUM") as pu,
            tc.tile_pool(name="pf", bufs=1, space="PSUM") as pf,
        ):
            # Resident constants.
            wk2 = cpool.tile([P, 2 * D], xdt)
            nc.sync.dma_start(out=wk2[:, 0:D], in_=wkt[0:P, :])
            nc.sync.dma_start(out=wk2[:, D : 2 * D], in_=wkt[P : 2 * P, :])
            wv2 = cpool.tile([P, 2 * D], xdt)
            nc.sync.dma_start(out=wv2[:, 0:D], in_=wv[0:P, :])
            nc.sync.dma_start(out=wv2[:, D : 2 * D], in_=wv[P : 2 * P, :])
            ident = cpool.tile([P, P], xdt)
            make_identity(nc, ident[:])
            zmat = cpool.tile([P, P], xdt)
            nc.vector.memset(zmat[:], 0.0)
            ones1 = cpool.tile([P, 1], xdt)
            nc.vector.memset(ones1[:], 1.0)
            one_f = cpool.tile([P, 1], f32)
            nc.vector.memset(one_f[:], 1.0)

            # Index arrays resident in SBUF.
            gx_all = cpool.tile([P, n_cols], i32)
            nc.sync.dma_start(
                out=gx_all[:], in_=gidx[:].rearrange("(p c) -> p c", c=n_cols)
            )
            # mask-quadratic lhsT rows at partition bases {0,32,64}
            slxT = cpool.tile([P, nwrap * P], xdt)
            for v in range(3):
                nc.sync.dma_start(
                    out=slxT[32 * v : 32 * v + 5, :],
                    in_=slxq[5 * v : 5 * v + 5, :],
                )
            rxt = cpool.tile([P, WMAX], xdt)
            nc.sync.dma_start(out=rxt[:], in_=rext[:])
            root_all = cpool.tile([P, n_blocks], i32)
            nc.sync.dma_start(
                out=root_all[:], in_=root[:].rearrange("(p b) -> p b", b=n_blocks)
            )

            # All root-row gathers up front (one 128-row gather per block:
            # HW indirect DMA honors exactly one index per partition).
            q_all = cpool.tile([P, n_blocks * D], xdt)
            for b0 in range(n_blocks):
                indirect_gather(
                    q_all[:, b0 * D : (b0 + 1) * D], root_all[:, b0 : b0 + 1]
                )

            # Gather groups: GB consecutive blocks per indirect DMA.
            groups = []
            b = 0
            while b < n_blocks:
                b2 = min(b + GB, n_blocks)
                groups.append((b, b2))
                b = b2

            xt_rot = kn["xt_pattern"]
            xt_i = 0

            def issue_gather(g0, g1):
                t_lo, t_hi = int(tile_off[g0]), int(tile_off[g1])
                xb = xgpool.tile([P, (t_hi - t_lo) * D], xdt, tag="xb")
                for c in range(t_lo, t_hi):
                    indirect_gather(
                        xb[:, (c - t_lo) * D : (c - t_lo + 1) * D],
                        gx_all[:, c : c + 1],
                    )
                return xb

            for _rep in range(reps):
              pending = {}
              for gi, (g0, g1) in enumerate(groups):
                # lookahead: keep the next group's gather ahead of this
                # group's Pool-queue copies (in-order engine queue)
                if gi not in pending:
                    pending[gi] = issue_gather(g0, g1)
                if gi + 1 < len(groups):
                    pending[gi + 1] = issue_gather(*groups[gi + 1])
                xb = pending.pop(gi)
                t_lo = int(tile_off[g0])
                for b in range(g0, g1):
                    t_b = int(tiles_per_block[b])
                    c0 = int(tile_off[b])
                    xoff = (c0 - t_lo) * D  # block's offset into xb

                    # ---- q side: qkT [d-chunks, segs] ----
                    qT_ps = pqt.tile([P, D], xdt, tag="qt")
                    for k in range(2):
                        nc.tensor.transpose(
                            qT_ps[:, k * P : (k + 1) * P],
                            q_all[:, b * D + k * P : b * D + (k + 1) * P],
                            ident[:],
                        )
                    qT = bpool.tile([P, D], xdt, tag="qT")
                    copy(kn["qt_copy"], qT[:], qT_ps[:, 0:D])
                    # combined PSUM tile: qkT_ps | s_ps slots
                    qs = pq.tile([P, D + t_b * WMAX], f32, tag="qs")
                    qkT_ps = qs[:, 0:D]
                    for m in range(2):
                        for k in range(2):
                            nc.tensor.matmul(
                                qkT_ps[:, m * P : (m + 1) * P],
                                lhsT=wk2[:, k * D + m * P : k * D + (m + 1) * P],
                                rhs=qT[:, k * P : (k + 1) * P],
                                start=(k == 0),
                                stop=(k == 1),
                            )
                    qkT = bpool.tile([P, D], xdt, tag="qkT")
                    copy(kn["qkt_copy"], qkT[:], qkT_ps[:, 0:D])

                    # ---- U^T psum init (covers pad/empty segment cols);
                    # col 384 holds the transposed Z column at finalize ----
                    u_psT = pu.tile([P, 2 * P + P + 1], f32, tag="u")
                    nc.tensor.matmul(
                        u_psT[:, 0 : 2 * P + P],
                        lhsT=zmat[:],
                        rhs=wk2[:, 0 : 2 * P + P],
                        start=True,
                        stop=False,
                        skip_group_check=True,
                    )

                    # ---- x transposes, batched TB tiles per PSUM bank ----
                    s_ps = qs[:, D : D + t_b * WMAX]
                    n_grp = math.ceil(t_b / TB)
                    xTg_list = []
                    for gi in range(n_grp):
                        tg0 = gi * TB
                        tg1 = min(tg0 + TB, t_b)
                        gw = (tg1 - tg0) * D
                        xT_ps = ptr.tile([P, TB * D], xdt, tag="tr")
                        for t in range(tg0, tg1):
                            for k in range(2):
                                nc.tensor.transpose(
                                    xT_ps[:, (t - tg0) * D + k * P :
                                          (t - tg0) * D + (k + 1) * P],
                                    xb[:, xoff + t * D + k * P :
                                       xoff + t * D + (k + 1) * P],
                                    ident[:],
                                )
                        xTg = xtpool.tile([P, TB * D], xdt, tag="xT")
                        code = xt_rot[xt_i % len(xt_rot)]
                        xt_i += 1
                        copy(code, xTg[:, 0:gw], xT_ps[:, 0:gw])
                        xTg_list.append((xTg, tg0, tg1))

                        # S for this group's tiles (full WMAX slots); the
                        # third matmul adds -32*(j - slid_e)^2 so exp() of
                        # off-diagonal entries is ~e^-32 (no mask op needed)
                        for t in range(tg0, tg1):
                            lo, w = windows[c0 + t]
                            c = c0 + t
                            v = 32 * (c % 3)
                            cb = c // 3
                            for k in range(2):
                                nc.tensor.matmul(
                                    s_ps[:, t * WMAX : (t + 1) * WMAX],
                                    lhsT=xTg[:, (t - tg0) * D + k * P :
                                             (t - tg0) * D + (k + 1) * P],
                                    rhs=qkT[:, k * P + lo : k * P + lo + WMAX],
                                    start=(k == 0),
                                    stop=False,
                                )
                            nc.tensor.matmul(
                                s_ps[:, t * WMAX : (t + 1) * WMAX],
                                lhsT=slxT[v : v + 5, cb * P : (cb + 1) * P],
                                rhs=rxt[v : v + 5, 0:WMAX],
                                start=False,
                                stop=True,
                            )
                        # exp for the whole group -> probabilities directly
                        ptil = etpool.tile([P, TB * WMAX], xdt, tag="pt")
                        if kn.get("exp_per_tile"):
                            for t in range(tg0, tg1):
                                nc.scalar.activation(
                                    ptil[:, (t - tg0) * WMAX : (t - tg0 + 1) * WMAX],
                                    s_ps[:, t * WMAX : (t + 1) * WMAX],
                                    EXP,
                                )
                        else:
                            nc.scalar.activation(
                                ptil[:, 0 : (tg1 - tg0) * WMAX],
                                s_ps[:, tg0 * WMAX : tg1 * WMAX],
                                EXP,
                            )
                        # U/Z accumulation per tile
                        for t in range(tg0, tg1):
                            lo, w = windows[c0 + t]
                            pt0 = (t - tg0) * WMAX
                            last = t == t_b - 1
                            for k in range(2):
                                nc.tensor.matmul(
                                    u_psT[:, k * P + lo : k * P + lo + w],
                                    lhsT=xb[:, xoff + t * D + k * P :
                                           xoff + t * D + (k + 1) * P],
                                    rhs=ptil[:, pt0 : pt0 + w],
                                    start=False,
                                    stop=last,
                                    skip_group_check=True,
                                )
                            nc.tensor.matmul(
                                u_psT[0:1, 2 * P + lo : 2 * P + lo + w],
                                lhsT=ones1[:],
                                rhs=ptil[:, pt0 : pt0 + w],
                                start=False,
                                stop=last,
                                skip_group_check=True,
                            )

                    # ---- finalize block ----
                    uT = bpool.tile([P, D], xdt, tag="uT")
                    copy(kn["ut_copy"], uT[:], u_psT[:, 0 : 2 * P])
                    zr = bpool.tile([P, P], f32, tag="zr")
                    copy(kn["zrow_copy"], zr[0:1, :], u_psT[0:1, 2 * P : 3 * P])
                    zT_ps = u_psT[:, 3 * P : 3 * P + 1]
                    nc.tensor.matmul(zT_ps, zr[0:1, :], one_f[0:1, 0:1],
                                     is_transpose=True, skip_group_check=True)
                    # eps 1e-8: way above the ~1e-10 quadratic-mask leakage
                    # (so empty segments divide to ~0, not garbage averages)
                    # and way below any real segment's z (>= exp(-18))
                    zm = bpool.tile([P, 1], f32, tag="zm")
                    nc.vector.tensor_scalar_max(zm[:], zT_ps[:], 1e-8)
                    rz = bpool.tile([P, 1], f32, tag="rz")
                    nc.vector.reciprocal(rz[:], zm[:])
                    f_ps = pf.tile([P, D], f32, tag="f")
                    for k in range(2):
                        nc.tensor.matmul(
                            f_ps[:],
                            lhsT=uT[:, k * P : (k + 1) * P],
                            rhs=wv2[:, k * D : (k + 1) * D],
                            start=(k == 0),
                            stop=(k == 1),
                        )
                    o = bpool.tile([P, D], f32, tag="o")
                    if kn["o_scale"] == "a":
                        nc.scalar.activation(o[:], f_ps[:], COPY, scale=rz[:, 0:1])
                    else:
                        nc.vector.tensor_scalar_mul(o[:], f_ps[:], rz[:, 0:1])
                    nc.sync.dma_start(out=out[b * P : (b + 1) * P, :], in_=o[:])
    nc.finalize()
    return nc


_NC_CACHE = {}


def _get_nc(n_ast, tiles_per_block, mode="full", reps=1, xp_bufs=None,
            blk_bufs=None, xdt=f32, windows=None, knobs=None):
    key = (n_ast, tuple(tiles_per_block), mode, reps, str(xdt),
           tuple(windows) if windows is not None else None,
           tuple(sorted((knobs or {}).items())))
    if key not in _NC_CACHE:
        _NC_CACHE[key] = _build_nc(
            n_ast, list(tiles_per_block), list(windows), reps=reps, xdt=xdt,
            knobs=knobs,
        )
    return _NC_CACHE[key]


def _binpack_core(counts_core, n_blocks, caps=None):
    """Bin-pack segments into n_blocks blocks of <=128 segs each.

    caps: per-block element capacities (desc order); best-fit decreasing.
    Returns (blocks, loads); blocks sorted by load desc, segments big/small
    interleaved inside each block so per-tile segment windows stay tight.
    """
    import heapq

    order = np.argsort(-counts_core, kind="stable")
    blocks = [[] for _ in range(n_blocks)]
    loads = np.zeros(n_blocks, dtype=np.int64)
    if caps is not None:
        caps = np.asarray(caps)
        heap = [(-caps[b], b) for b in range(n_blocks)]
        heapq.heapify(heap)
        for s in order:
            load, b = heapq.heappop(heap)
            rem = -load
            n_e = counts_core[s]
            if n_e > rem:
                raise OverflowError((b, n_e, rem))
            blocks[b].append(s)
            loads[b] += n_e
            if len(blocks[b]) < P:
                heapq.heappush(heap, (-(rem - n_e), b))
        border = np.argsort(-loads, kind="stable")
        blocks = [blocks[i] for i in border]
        loads = loads[border]
    else:
        heap = [(0, b) for b in range(n_blocks)]
        heapq.heapify(heap)
        for s in order:
            load, b = heapq.heappop(heap)
            blocks[b].append(s)
            loads[b] = load + counts_core[s]
            if len(blocks[b]) < P:
                heapq.heappush(heap, (loads[b], b))
        border = np.argsort(-loads, kind="stable")
        blocks = [blocks[i] for i in border]
        loads = loads[border]
    inter = []
    for segs in blocks:
        segs = sorted(segs, key=lambda s: -counts_core[s])
        out, i, j = [], 0, len(segs) - 1
        while i <= j:
            out.append(segs[i])
            if i < j:
                out.append(segs[j])
            i += 1
            j -= 1
        inter.append(out)
    return inter, loads


def prepare_in_maps(
    ast_np, wkt_s, wv_np, ast_to_pdg_key, ast_to_pdg_value,
    pdg_to_root_key, pdg_to_root_value, n_pdg,
    segs_per_core=None, n_blocks=None, x_np_dt=None,
):
    """Host-side prep: sort elements by segment, bin-pack segments into
    blocks (per core), pad, build per-core in_maps.

    Returns (in_maps, meta); meta["windows"][c] = (lo, w) true local-segment
    window of tile c (cross-core union), meta["out_seg"] maps out rows to
    global segment ids.
    """
    n_ast = ast_np.shape[0]
    if x_np_dt is None:
        x_np_dt = X_NP_DT
    if ast_np.dtype != x_np_dt:
        ast_np = np.ascontiguousarray(ast_np.astype(x_np_dt))
        wkt_s = np.ascontiguousarray(wkt_s.astype(x_np_dt))
        wv_np = np.ascontiguousarray(wv_np.astype(x_np_dt))
    if segs_per_core is None:
        segs_per_core = (n_pdg + N_CORES - 1) // N_CORES
    if n_blocks is None:
        n_blocks = math.ceil(segs_per_core / P)

    order = np.argsort(ast_to_pdg_value, kind="stable")
    seg_sorted = np.asarray(ast_to_pdg_value)[order]
    gid_sorted = np.asarray(ast_to_pdg_key)[order].astype(np.int32)
    counts = np.bincount(seg_sorted, minlength=n_pdg).astype(np.int64)
    cum = np.concatenate([[0], np.cumsum(counts)]).astype(np.int64)

    root_full = np.zeros(n_pdg, dtype=np.int32)
    root_full[np.asarray(pdg_to_root_key)] = np.asarray(pdg_to_root_value)

    core_E = np.array([
        int(counts[c * segs_per_core : min((c + 1) * segs_per_core, n_pdg)]
            .sum()) for c in range(N_CORES)
    ])
    core_blocks = None
    for slack in (1.005, 1.02, 1.05, 1.15):
        need = int(core_E.max() * slack) + 2 * P
        base = need // n_blocks // P
        n_hi = min(n_blocks,
                   math.ceil((need - n_blocks * base * P) / P))
        tiles_per_block = np.array(
            [base + 1] * n_hi + [base] * (n_blocks - n_hi), dtype=int)
        caps = tiles_per_block * P
        try:
            core_blocks = [
                _binpack_core(
                    counts[c * segs_per_core : min((c + 1) * segs_per_core,
                                                   n_pdg)],
                    n_blocks, caps=caps)[0]
                for c in range(N_CORES)
            ]
            break
        except OverflowError:
            continue
    assert core_blocks is not None, "bin packing failed at all slacks"
    tile_off = np.concatenate([[0], np.cumsum(tiles_per_block)]).astype(int)
    n_cols = int(tile_off[-1])
    seg_slots = n_blocks * P
    e_slots = n_cols * P

    in_maps = []
    out_seg = []
    core_arrays = []
    win_lo = np.full(n_cols, P, dtype=np.int64)   # cross-core min
    win_hi = np.zeros(n_cols, dtype=np.int64)     # cross-core max
    for c in range(N_CORES):
        s0 = c * segs_per_core
        gidx_core = np.zeros(e_slots, dtype=np.int32)
        slid_core = np.full(e_slots, -1.0, dtype=np.float64)
        root_core = np.zeros(seg_slots, dtype=np.int32)
        oseg = np.full(seg_slots, -1, dtype=np.int64)
        for b, segs in enumerate(core_blocks[c]):
            o0 = tile_off[b] * P
            cap = tiles_per_block[b] * P
            pos = 0
            for j, sl in enumerate(segs):
                g = s0 + sl
                root_core[b * P + j] = root_full[g]
                oseg[b * P + j] = g
                e0, e1 = cum[g], cum[g + 1]
                n_e = e1 - e0
                if pos + n_e > cap:
                    raise OverflowError((c, b, pos, n_e, cap))
                gidx_core[o0 + pos : o0 + pos + n_e] = gid_sorted[e0:e1]
                slid_core[o0 + pos : o0 + pos + n_e] = float(j)
                pos += n_e
        sl2 = slid_core.reshape(n_cols, P)
        for t in range(n_cols):
            v = sl2[t][sl2[t] >= 0]
            if len(v):
                win_lo[t] = min(win_lo[t], int(v.min()))
                win_hi[t] = max(win_hi[t], int(v.max()) + 1)
        core_arrays.append((gidx_core, slid_core, root_core))
        out_seg.append(oseg)

    # true windows (lo > 0 fine: all windowed writes are free-dim); clamp
    # lo so lo + WMAX <= 128 (S always reads a full WMAX-wide qkT slice)
    win_lo = np.minimum(win_lo, P)
    win_w = np.maximum(win_hi - win_lo, 1)
    wmax = int(win_w.max())
    win_lo = np.minimum(win_lo, P - wmax)
    win_w = np.maximum(win_hi - win_lo, 1)
    assert win_w.max() <= wmax
    assert wmax <= 32, f"wmax={wmax}"
    windows = tuple((int(win_lo[t]), int(win_w[t])) for t in range(n_cols))

    def _bf16_split(vals):
        """vals -> (hi, lo) with hi+lo == vals and both bf16-exact."""
        hi = np.asarray(vals, np.float64).astype(x_np_dt).astype(np.float64)
        lo = np.asarray(vals, np.float64) - hi
        assert np.array_equal(lo.astype(x_np_dt).astype(np.float64), lo)
        return hi, lo

    # rext: rhs rows of the -32*(j - s)^2 quadratic, replicated at the
    # legal partition bases {0,32,64}: [-32, -32, 64j, (-32j^2)_hi, _lo].
    # M=32 is a power of two and wide values are hi/lo split, so every
    # bf16 coefficient (and its fp32 product) is exact.
    j = np.arange(wmax, dtype=np.float64)
    jq_hi, jq_lo = _bf16_split(-32.0 * j * j)
    rext = np.zeros((P, wmax), dtype=np.float64)
    for v in range(3):
        rext[32 * v + 0] = -32.0
        rext[32 * v + 1] = -32.0
        rext[32 * v + 2] = 64.0 * j
        rext[32 * v + 3] = jq_hi
        rext[32 * v + 4] = jq_lo
    rext = np.ascontiguousarray(rext.astype(x_np_dt))

    nwrap = math.ceil(n_cols / 3)
    for c in range(N_CORES):
        gidx_core, slid_core, root_core = core_arrays[c]
        sl2 = slid_core.reshape(n_cols, P)
        for t in range(n_cols):
            sl2[t] -= win_lo[t]  # pads (-1) go further negative: suppressed
        # slxq rows [s2_hi, s2_lo, s, 1, 1] per tile, compact [15, nwrap*128]:
        # tile c -> rows 5*(c%3)+r, cols (c//3)*128 + e
        slxq = np.zeros((15, nwrap * P), dtype=np.float64)
        for t in range(n_cols):
            r0 = 5 * (t % 3)
            cc = (t // 3) * P
            s = sl2[t]
            sq_hi, sq_lo = _bf16_split(s * s)
            slxq[r0 + 0, cc : cc + P] = sq_hi
            slxq[r0 + 1, cc : cc + P] = sq_lo
            slxq[r0 + 2, cc : cc + P] = s
            slxq[r0 + 3, cc : cc + P] = 1.0
            slxq[r0 + 4, cc : cc + P] = 1.0
        slxq = np.ascontiguousarray(slxq.astype(x_np_dt))
        gidx_core = np.ascontiguousarray(
            gidx_core.reshape(n_cols, P).T).ravel()
        root_core = np.ascontiguousarray(
            root_core.reshape(n_blocks, P).T).ravel()
        in_maps.append({
            "ast": ast_np,
            "wkt": wkt_s,
            "wv": wv_np,
            "gidx": gidx_core,
            "slxq": slxq,
            "rext": rext,
            "root": root_core,
        })

    meta = {
        "windows": windows,
        "wmax": wmax,
        "empty_segs": np.where(counts == 0)[0],
        "x_np_dt": x_np_dt,
        "n_ast": n_ast,
        "n_blocks": n_blocks,
        "tiles_per_block": tiles_per_block,
        "segs_per_core": segs_per_core,
        "n_pdg": n_pdg,
        "out_seg": out_seg,
        "n_tiles_total": n_cols,
    }
    return in_maps, meta


def _run(
    ast_np, wkt_s, wv_np, ast_to_pdg_key, ast_to_pdg_value,
    pdg_to_root_key, pdg_to_root_value, n_pdg,
    segs_per_core=None, n_blocks=None, trace=False, knobs=None,
):
    in_maps, meta = prepare_in_maps(
        ast_np, wkt_s, wv_np, ast_to_pdg_key, ast_to_pdg_value,
        pdg_to_root_key, pdg_to_root_value, n_pdg,
        segs_per_core=segs_per_core, n_blocks=n_blocks,
    )
    nc = _get_nc(meta["n_ast"], meta["tiles_per_block"], xdt=X_DT,
                 windows=meta["windows"], knobs=knobs)
    res = run_bass_kernel_spmd(nc, in_maps, list(range(N_CORES)), trace=trace)

    full = np.zeros((n_pdg, D), dtype=np.float32)
    for c in range(N_CORES):
        oseg = meta["out_seg"][c]
        valid = oseg >= 0
        full[oseg[valid]] = res.results[c]["out"][valid]
    # segments with no elements: reference yields exact zeros
    full[meta["empty_segs"]] = 0.0
    return full, res


def kernel(
    ast_nodes_encodings, Wk, Wv, ast_to_pdg_key, ast_to_pdg_value,
    pdg_to_root_key, pdg_to_root_value, nr_cfg_nodes,
):
    ast_np = np.ascontiguousarray(np.asarray(ast_nodes_encodings, dtype=np.float32))
    wk_np = np.asarray(Wk, dtype=np.float32)
    wv_np = np.ascontiguousarray(np.asarray(Wv, dtype=np.float32))
    scale = np.float32(1.0 / np.sqrt(ast_np.shape[1]))
    wkt_s = np.ascontiguousarray(wk_np.T * scale)

    n_pdg = int(nr_cfg_nodes)
    assert ast_np.shape == (N_AST_FULL, D) and n_pdg == N_PDG_FULL

    full, _ = _run(
        ast_np, wkt_s, wv_np,
        np.asarray(ast_to_pdg_key), np.asarray(ast_to_pdg_value),
        np.asarray(pdg_to_root_key), np.asarray(pdg_to_root_value),
        n_pdg,
    )
    return full
